# revision 1
# baseline (speedup 1.0000x reference)
"""GNN message-passing kernel for Trainium2 (Bass/Tile), 8-core SPMD.

Sharding: edges sharded by receiver range (edge/data parallel, no collectives).
Core c owns receivers in [c*NPC, (c+1)*NPC). Each core:
  phase A: P1  = nf @ W1            (full table, all cores identical)
  phase B: P2b = nf_loc @ W2 + b    (local shard only)
  phase C: per 128-node window, per 128-edge tile:
           msg = relu(P1[s] + P2b[r] + efT.T @ W3)
           aggr[window] += S_tile.T @ msg          (S = one-hot of ranks)
           out = LayerNorm(aggr + nf_shard)
All matmuls/adds in fp32 (bit-accuracy limited only by summation order).
"""

import numpy as np

import concourse.bacc as bacc
import concourse.tile as tile
import concourse.mybir as mybir
import concourse.bass as bass
from concourse.tile_rust import add_dep_helper

F32 = mybir.dt.float32
I16 = mybir.dt.int16


# ----------------------------------------------------------------------------
# Host-side preparation
# ----------------------------------------------------------------------------

def host_prep(node_features, senders, receivers, edge_features, W, b, ln_w, ln_b,
              n_cores=8, hi_base=32768):
    N, H = node_features.shape
    E = senders.shape[0]
    assert H == 128
    NPC = N // n_cores                      # nodes per core
    WPC = (NPC + 127) // 128                # windows per core
    NPC_PAD = WPC * 128
    NT_GLOBAL = (N + 127) // 128            # node tiles for P1 table
    N_PAD = NT_GLOBAL * 128
    HI_BASE = hi_base                       # int16 gather split point

    node_features = np.asarray(node_features, np.float32)
    senders = np.asarray(senders, np.int32)
    receivers = np.asarray(receivers, np.int32)
    edge_features = np.asarray(edge_features, np.float32)

    core_of_edge = receivers // NPC
    core_of_edge = np.minimum(core_of_edge, n_cores - 1)  # guard (shouldn't hit)

    # ---- pass 1: per-core, per-window lo/hi counts --------------------------
    per_core = []
    lo_cnt = np.zeros((n_cores, WPC), np.int64)
    hi_cnt = np.zeros((n_cores, WPC), np.int64)
    for c in range(n_cores):
        sel = np.nonzero(core_of_edge == c)[0]
        r_loc = receivers[sel] - c * NPC
        s = senders[sel]
        w = r_loc // 128
        hi = (s >= HI_BASE)
        order = np.lexsort((hi, w))
        sel, r_loc, s, w, hi = sel[order], r_loc[order], s[order], w[order], hi[order]
        lo_cnt[c] = np.bincount(w[~hi], minlength=WPC)
        hi_cnt[c] = np.bincount(w[hi], minlength=WPC)
        per_core.append((sel, r_loc, s, w, hi))

    T_lo = ((lo_cnt.max(axis=0) + 127) // 128).astype(np.int64)
    T_hi = ((hi_cnt.max(axis=0) + 127) // 128).astype(np.int64)
    T_w = T_lo + T_hi                       # tiles per window (shared all cores)
    NT = int(T_w.sum())                     # total tiles per core
    E_PAD = NT * 128

    # window tile-base offsets in the padded stream (tile units)
    tile_base = np.concatenate(([0], np.cumsum(T_w)[:-1]))
    lo_tile_base = np.concatenate(([0], np.cumsum(T_lo)[:-1]))
    hi_tile_base = np.concatenate(([0], np.cumsum(T_hi)[:-1]))
    L_LO = int(T_lo.sum()) * 128
    L_HI = max(int(T_hi.sum()) * 128, 128)

    def wrap_idx(arr):
        """int16 stream -> [128, L/16] wrapped layout (replicated per 16 rows)."""
        L = arr.shape[0]
        assert L % 16 == 0
        w16 = arr.reshape(-1, 16).T.astype(np.int16)   # [16, L/16]
        return np.ascontiguousarray(np.tile(w16, (8, 1)))

    structure = dict(N=N, H=H, E=E, NPC=NPC, WPC=WPC, NPC_PAD=NPC_PAD,
                     NT_GLOBAL=NT_GLOBAL, N_PAD=N_PAD, NT=NT, E_PAD=E_PAD,
                     T_lo=T_lo, T_hi=T_hi, T_w=T_w, tile_base=tile_base,
                     lo_tile_base=lo_tile_base, hi_tile_base=hi_tile_base,
                     L_LO=L_LO, L_HI=L_HI, HI_BASE=HI_BASE)

    # ---- shared (core-independent) inputs -----------------------------------
    nfT = np.zeros((128, N_PAD), np.float32)
    nfT[:, :N] = node_features.T
    iota_row = np.broadcast_to(np.arange(128, dtype=np.float32), (128, 128)).copy()
    shared = {
        "nfT": nfT,
        "W1": np.ascontiguousarray(W[0:128], np.float32),
        "W2": np.ascontiguousarray(W[128:256], np.float32),
        "W3": np.ascontiguousarray(W[256:384], np.float32),
        "b_bc": np.broadcast_to(np.asarray(b, np.float32), (128, 128)).copy(),
        "b_col": np.asarray(b, np.float32).reshape(128, 1).copy(),
        "ident": np.eye(128, dtype=np.float32),
        "lnw_bc": np.broadcast_to(np.asarray(ln_w, np.float32), (128, 128)).copy(),
        "lnb_bc": np.broadcast_to(np.asarray(ln_b, np.float32), (128, 128)).copy(),
        "iota": iota_row,
    }

    # ---- pass 2: per-core padded streams ------------------------------------
    in_maps = []
    for c in range(n_cores):
        sel, r_loc, s, w, hi = per_core[c]
        Ec = sel.shape[0]
        # within-window within-group running index
        grp = w * 2 + hi.astype(np.int64)            # sorted by (w, hi)
        starts = np.concatenate(([0], np.nonzero(np.diff(grp))[0] + 1))
        grp_start_per_edge = np.repeat(starts, np.diff(np.concatenate((starts, [Ec]))))
        j = np.arange(Ec) - grp_start_per_edge
        pos = np.where(
            hi,
            (tile_base[w] + T_lo[w]) * 128 + j,
            tile_base[w] * 128 + j,
        )

        ef_pad = np.zeros((E_PAD, 128), np.float32)
        ef_pad[pos] = edge_features[sel]
        efT = np.ascontiguousarray(ef_pad.T)

        rank = np.full(E_PAD, -1.0, np.float32)
        rank[pos] = (r_loc - w * 128).astype(np.float32)
        rankT = np.ascontiguousarray(rank.reshape(NT, 128).T)   # [128, NT]

        idx_lo = np.zeros(L_LO, np.int64)
        lo_pos = lo_tile_base[w[~hi]] * 128 + j[~hi]
        idx_lo[lo_pos] = s[~hi]
        idx_hi = np.zeros(L_HI, np.int64)
        hi_pos = hi_tile_base[w[hi]] * 128 + j[hi]
        idx_hi[hi_pos] = s[hi] - HI_BASE
        idx_r = np.zeros(E_PAD, np.int64)
        idx_r[pos] = r_loc
        idx_rank = np.zeros(E_PAD, np.int64)
        idx_rank[pos] = r_loc - w * 128

        nf_shard = np.zeros((NPC_PAD, 128), np.float32)
        nf_shard[:NPC] = node_features[c * NPC:(c + 1) * NPC]
        nfT_loc = np.zeros((128, NPC_PAD), np.float32)
        nfT_loc[:, :NPC] = node_features[c * NPC:(c + 1) * NPC].T

        m = dict(shared)
        m.update({
            "efT": efT,
            "rankT": rankT,
            "idx_lo": wrap_idx(idx_lo),
            "idx_hi": wrap_idx(idx_hi),
            "idx_r": wrap_idx(idx_r),
            "idx_rank": wrap_idx(idx_rank),
            "nf_shard": nf_shard,
            "nfT_loc": np.ascontiguousarray(nfT_loc),
        })
        in_maps.append(m)

    return structure, in_maps


# ----------------------------------------------------------------------------
# Bass kernel builder
# ----------------------------------------------------------------------------

def _emit_ln_store(nc, tc, wtiles, x, eps_sb, lnw_sb, lnb_sb, out_shard, w):
    """LayerNorm(x) * ln_w + ln_b -> out_shard[w*128:(w+1)*128]."""
    stats = wtiles.tile([128, 6], F32, tag="stats")
    nc.vector.bn_stats(out=stats[:], in_=x[:])
    mv = wtiles.tile([128, 2], F32, tag="mv")
    nc.vector.bn_aggr(out=mv[:], in_=stats[:])
    sd = wtiles.tile([128, 1], F32, tag="sd")
    nc.scalar.activation(
        out=sd[:], in_=mv[:, 1:2],
        func=mybir.ActivationFunctionType.Sqrt,
        bias=eps_sb[:], scale=1.0)
    rs = wtiles.tile([128, 1], F32, tag="rs")
    nc.vector.reciprocal(out=rs[:], in_=sd[:])
    xn = wtiles.tile([128, 128], F32, tag="xn")
    nc.vector.tensor_scalar(
        out=xn[:], in0=x[:], scalar1=mv[:, 0:1], scalar2=rs[:],
        op0=mybir.AluOpType.subtract, op1=mybir.AluOpType.mult)
    xw = wtiles.tile([128, 128], F32, tag="xw")
    nc.vector.tensor_mul(out=xw[:], in0=xn[:], in1=lnw_sb[:])
    ot = wtiles.tile([128, 128], F32, tag="ot")
    nc.vector.tensor_add(out=ot[:], in0=xw[:], in1=lnb_sb[:])
    nc.sync.dma_start(out=out_shard[w * 128:(w + 1) * 128, :], in_=ot[:])


def build_kernel(st, eps=1e-5, max_windows=None, use_gathers=True,
                 use_edge_mm=True, use_agg=True):
    N_PAD, NT_GLOBAL = st["N_PAD"], st["NT_GLOBAL"]
    NPC_PAD, WPC = st["NPC_PAD"], st["WPC"]
    NT, E_PAD = st["NT"], st["E_PAD"]
    T_lo, T_hi, T_w = st["T_lo"], st["T_hi"], st["T_w"]
    tile_base, lo_tile_base, hi_tile_base = (
        st["tile_base"], st["lo_tile_base"], st["hi_tile_base"])
    L_LO, L_HI, HI_BASE = st["L_LO"], st["L_HI"], st["HI_BASE"]
    T_MAX = int(T_w.max())

    nc = bacc.Bacc("TRN2", target_bir_lowering=False, debug=False)

    # inputs
    nfT = nc.dram_tensor("nfT", [128, N_PAD], F32, kind="ExternalInput")
    nfT_loc = nc.dram_tensor("nfT_loc", [128, NPC_PAD], F32, kind="ExternalInput")
    efT = nc.dram_tensor("efT", [128, E_PAD], F32, kind="ExternalInput")
    rankT = nc.dram_tensor("rankT", [128, NT], F32, kind="ExternalInput")
    idx_lo = nc.dram_tensor("idx_lo", [128, L_LO // 16], I16, kind="ExternalInput")
    idx_hi = nc.dram_tensor("idx_hi", [128, L_HI // 16], I16, kind="ExternalInput")
    idx_r = nc.dram_tensor("idx_r", [128, E_PAD // 16], I16, kind="ExternalInput")
    idx_rank = nc.dram_tensor("idx_rank", [128, E_PAD // 16], I16,
                              kind="ExternalInput")
    nf_shard = nc.dram_tensor("nf_shard", [NPC_PAD, 128], F32, kind="ExternalInput")
    W1 = nc.dram_tensor("W1", [128, 128], F32, kind="ExternalInput")
    W2 = nc.dram_tensor("W2", [128, 128], F32, kind="ExternalInput")
    W3 = nc.dram_tensor("W3", [128, 128], F32, kind="ExternalInput")
    b_bc = nc.dram_tensor("b_bc", [128, 128], F32, kind="ExternalInput")
    b_col = nc.dram_tensor("b_col", [128, 1], F32, kind="ExternalInput")
    ident = nc.dram_tensor("ident", [128, 128], F32, kind="ExternalInput")
    lnw_bc = nc.dram_tensor("lnw_bc", [128, 128], F32, kind="ExternalInput")
    lnb_bc = nc.dram_tensor("lnb_bc", [128, 128], F32, kind="ExternalInput")
    iota_in = nc.dram_tensor("iota", [128, 128], F32, kind="ExternalInput")

    # internal scratch + output
    P1 = nc.dram_tensor("P1", [N_PAD, 128], F32, kind="Internal")
    out_shard = nc.dram_tensor("out_shard", [NPC_PAD, 128], F32,
                               kind="ExternalOutput")

    with tile.TileContext(nc) as tc:
        with (
            tc.tile_pool(name="consts", bufs=1) as consts,
            tc.tile_pool(name="ptiles", bufs=4) as ptiles,
            tc.tile_pool(name="ppsum", bufs=4, space="PSUM") as ppsum,
            tc.tile_pool(name="ppsumb", bufs=2, space="PSUM") as ppsumb,
            tc.tile_pool(name="gtiles", bufs=3) as gtiles,
            tc.tile_pool(name="etile", bufs=3) as etile,
            tc.tile_pool(name="msgs", bufs=5) as msgs,
            tc.tile_pool(name="aggp", bufs=2, space="PSUM") as aggp,
            tc.tile_pool(name="wtiles", bufs=3) as wtiles,
        ):
            # constants in SBUF
            W1_sb = consts.tile([128, 128], F32)
            W2_sb = consts.tile([128, 128], F32)
            W3_sb = consts.tile([128, 128], F32)
            b_sb = consts.tile([128, 128], F32)
            lnw_sb = consts.tile([128, 128], F32)
            lnb_sb = consts.tile([128, 128], F32)
            iota_sb = consts.tile([128, 128], F32)
            bcol_sb = consts.tile([128, 1], F32)
            ident_sb = consts.tile([128, 128], F32)
            p2bT_sb = consts.tile([128, NPC_PAD], F32)
            eps_sb = consts.tile([128, 1], F32)
            for dst, src in ((W1_sb, W1), (W2_sb, W2), (W3_sb, W3),
                             (b_sb, b_bc), (lnw_sb, lnw_bc), (lnb_sb, lnb_bc),
                             (iota_sb, iota_in), (bcol_sb, b_col),
                             (ident_sb, ident)):
                nc.sync.dma_start(out=dst[:], in_=src[:])
            nc.vector.memset(eps_sb[:], eps)

            idxlo_sb = consts.tile([128, L_LO // 16], I16)
            idxhi_sb = consts.tile([128, L_HI // 16], I16)
            idxrank_sb = consts.tile([128, E_PAD // 16], I16)
            rankT_sb = consts.tile([128, NT], F32)
            nc.sync.dma_start(out=idxlo_sb[:], in_=idx_lo[:])
            nc.sync.dma_start(out=idxhi_sb[:], in_=idx_hi[:])
            nc.sync.dma_start(out=idxrank_sb[:], in_=idx_rank[:])
            nc.sync.dma_start(out=rankT_sb[:], in_=rankT[:])

            # ---------------- phase A: P1 = nf @ W1 (full, to DRAM) ----------
            for i0 in range(0, NT_GLOBAL, 4):
                k = min(4, NT_GLOBAL - i0)
                nf_t = ptiles.tile([128, 4 * 128], F32, tag="nf_t")
                nc.sync.dma_start(
                    out=nf_t[:, :k * 128],
                    in_=nfT[:, i0 * 128:(i0 + k) * 128])
                ps = ppsum.tile([128, 4, 128], F32, tag="pp")
                for t in range(k):
                    nc.tensor.matmul(
                        out=ps[:, t, :],
                        lhsT=nf_t[:, t * 128:(t + 1) * 128],
                        rhs=W1_sb[:],
                        start=True, stop=True)
                res = ptiles.tile([128, 4, 128], F32, tag="res")
                nc.scalar.copy(out=res[:, :k, :], in_=ps[:, :k, :])
                dst_ap = P1[i0 * 128:(i0 + k) * 128, :].rearrange(
                    "(t p) h -> p t h", p=128)
                nc.sync.dma_start(out=dst_ap, in_=res[:, :k, :])

            # ------ phase B: P2bT = (nf_loc @ W2 + b).T, kept in SBUF --------
            for j0 in range(0, NPC_PAD, 512):
                k = min(512, NPC_PAD - j0)
                nfl_t = ptiles.tile([128, 512], F32, tag="nfl_t")
                nc.sync.dma_start(out=nfl_t[:, :k], in_=nfT_loc[:, j0:j0 + k])
                psb = ppsumb.tile([128, 512], F32, tag="ppb")
                nc.tensor.matmul(
                    out=psb[:, :k], lhsT=W2_sb[:], rhs=nfl_t[:, :k],
                    start=True, stop=True)
                nc.scalar.add(out=p2bT_sb[:, j0:j0 + k], in_=psb[:, :k],
                              add=bcol_sb[:])

            # ---------------- phase C: edge loop -----------------------------
            n_win = WPC if max_windows is None else min(max_windows, WPC)
            for w in range(n_win):
                tw, tlo, thi = int(T_w[w]), int(T_lo[w]), int(T_hi[w])
                tb = int(tile_base[w])
                if tw == 0:
                    nf_w = wtiles.tile([128, 128], F32, tag="nfw")
                    nc.sync.dma_start(
                        out=nf_w[:], in_=nf_shard[w * 128:(w + 1) * 128, :])
                    x = wtiles.tile([128, 128], F32, tag="x")
                    nc.vector.tensor_copy(out=x[:], in_=nf_w[:])
                    _emit_ln_store(nc, tc, wtiles, x, eps_sb, lnw_sb, lnb_sb,
                                   out_shard, w)
                    continue
                g1 = gtiles.tile([128, T_MAX, 128], F32, tag="g1")
                if not use_gathers:
                    nc.vector.memset(g1[:, :tw, :], 0.0)
                if use_gathers and tlo > 0:
                    lb = int(lo_tile_base[w]) * 8   # 128/16 cols per tile
                    nc.gpsimd.dma_gather(
                        out_ap=g1[:, 0:tlo, :],
                        in_ap=P1[:, :],
                        idxs_ap=idxlo_sb[:, lb:lb + tlo * 8],
                        num_idxs=tlo * 128,
                        num_idxs_reg=tlo * 128,
                        elem_size=128, single_packet=False)
                if use_gathers and thi > 0:
                    hb = int(hi_tile_base[w]) * 8
                    nc.gpsimd.dma_gather(
                        out_ap=g1[:, tlo:tw, :],
                        in_ap=P1[HI_BASE:, :],
                        idxs_ap=idxhi_sb[:, hb:hb + thi * 8],
                        num_idxs=thi * 128,
                        num_idxs_reg=thi * 128,
                        elem_size=128, single_packet=False)
                p2x = gtiles.tile([128, T_MAX * 128], F32, tag="p2x")
                nc.gpsimd.ap_gather(
                    out_ap=p2x[:, :tw * 128].rearrange("p (n d) -> p n d", d=1),
                    in_ap=p2bT_sb[:, w * 128:(w + 1) * 128].rearrange(
                        "p (n d) -> p n d", d=1),
                    idxs_ap=idxrank_sb[:, tb * 8:(tb + tw) * 8],
                    channels=128, num_elems=128, d=1,
                    num_idxs=tw * 128)
                ef_sb = etile.tile([128, T_MAX * 128], F32, tag="ef")
                nc.sync.dma_start(
                    out=ef_sb[:, :tw * 128],
                    in_=efT[:, tb * 128:(tb + tw) * 128])

                agg = aggp.tile([128, 128], F32, tag="agg")
                if not use_agg:
                    nc.vector.memset(agg[:], 0.0)
                t_done = 0
                for c0 in range(0, tw, 4):
                    k = min(4, tw - c0)
                    pre = msgs.tile([128, 4, 128], F32, tag="pre")
                    for t in range(k):
                        ps = ppsum.tile([128, 128], F32, tag="pp")
                        if use_edge_mm:
                            # P2[r] lands first via PE transpose of the ap_gather
                            nc.tensor.matmul(
                                out=ps[:],
                                lhsT=p2x[:, (c0 + t) * 128:(c0 + t + 1) * 128],
                                rhs=ident_sb[:],
                                is_transpose=True,
                                start=True, stop=False,
                                skip_group_check=True)
                            nc.tensor.matmul(
                                out=ps[:],
                                lhsT=ef_sb[:, (c0 + t) * 128:(c0 + t + 1) * 128],
                                rhs=W3_sb[:],
                                start=False, stop=True,
                                skip_group_check=True)
                        else:
                            nc.vector.memset(ps[:], 0.0)
                        # pre = g1 + (EW + P2x)
                        nc.vector.tensor_add(
                            out=pre[:, t, :], in0=g1[:, c0 + t, :], in1=ps[:])
                    # msg = relu(pre)
                    msg = msgs.tile([128, 4, 128], F32, tag="msg")
                    nc.vector.tensor_scalar_max(
                        out=msg[:, :k, :], in0=pre[:, :k, :], scalar1=0.0)
                    # S one-hot + aggregation matmuls
                    S = msgs.tile([128, 4, 128], F32, tag="S")
                    iota_ap = iota_sb[:]
                    iota_bc = bass.AP(
                        tensor=iota_ap.tensor, offset=iota_ap.offset,
                        ap=[iota_ap.ap[0], [0, k], iota_ap.ap[1]])
                    rank_sl = rankT_sb[:, tb + c0:tb + c0 + k]
                    rank_bc = bass.AP(
                        tensor=rank_sl.tensor, offset=rank_sl.offset,
                        ap=[rank_sl.ap[0], rank_sl.ap[1], [0, 128]])
                    nc.vector.tensor_tensor(
                        out=S[:, :k, :], in0=iota_bc, in1=rank_bc,
                        op=mybir.AluOpType.is_equal)
                    if use_agg:
                        for t in range(k):
                            nc.tensor.matmul(
                                out=agg[:],
                                lhsT=S[:, t, :],
                                rhs=msg[:, t, :],
                                start=(t_done == 0), stop=(t_done == tw - 1),
                                skip_group_check=True)
                            t_done += 1

                # residual + LayerNorm
                nf_w = wtiles.tile([128, 128], F32, tag="nfw")
                nc.sync.dma_start(out=nf_w[:], in_=nf_shard[w * 128:(w + 1) * 128, :])
                x = wtiles.tile([128, 128], F32, tag="x")
                nc.vector.tensor_add(out=x[:], in0=agg[:], in1=nf_w[:])
                _emit_ln_store(nc, tc, wtiles, x, eps_sb, lnw_sb, lnb_sb,
                               out_shard, w)

    nc.compile()
    return nc


# ----------------------------------------------------------------------------
# Full entry: host prep + device run + assembly
# ----------------------------------------------------------------------------

def run(node_features, senders, receivers, edge_features, W, b, ln_w, ln_b,
        n_cores=8, return_nc=False):
    from concourse.bass_utils import run_bass_kernel_spmd
    st, in_maps = host_prep(node_features, senders, receivers, edge_features,
                            W, b, ln_w, ln_b, n_cores)
    nc = build_kernel(st)
    res = run_bass_kernel_spmd(nc, in_maps, core_ids=list(range(n_cores)))
    NPC = st["NPC"]
    out = np.concatenate(
        [res.results[c]["out_shard"][:NPC] for c in range(n_cores)], axis=0)
    if return_nc:
        return out, nc, st, in_maps
    return out


# ----------------------------------------------------------------------------
# Harness entry point
# ----------------------------------------------------------------------------

def kernel(**inputs):
    """Full-input entry: shards across 8 NeuronCores internally."""
    out = run(
        node_features=np.asarray(inputs["node_features"], np.float32),
        senders=np.asarray(inputs["senders"], np.int32),
        receivers=np.asarray(inputs["receivers"], np.int32),
        edge_features=np.asarray(inputs["edge_features"], np.float32),
        W=np.asarray(inputs["W"], np.float32),
        b=np.asarray(inputs["b"], np.float32),
        ln_w=np.asarray(inputs["ln_w"], np.float32),
        ln_b=np.asarray(inputs["ln_b"], np.float32),
        n_cores=8,
    )
    return out.astype(np.float32)



# revision 3
# speedup vs baseline: 2.4155x; 2.4155x over previous
"""GNN message-passing kernel for Trainium2 (Bass/Tile), 8-core SPMD.

Sharding: edges sharded by receiver range (edge/data parallel, no collectives).
Core c owns receivers in [c*NPC, (c+1)*NPC). Host prep (pure indexing, no
FLOPs) expands nf[senders] into a per-edge stream so the device never does a
DRAM gather on the sender side.

Per core, per 128-receiver window, per 128-edge tile (all in one PSUM group):
  ps  = transpose(p2x)            # P2b[r] via window-local ap_gather (f32r)
      + nf_exp_tile.T @ W1        # bf16, host-expanded nf[senders]
      + ef_tile.T    @ W3         # bf16
  msg = relu(ps)                  # Activation engine, bf16 out
  S   = (iota == rank)            # one-hot, DVE tensor_scalar, bf16
  agg += S.T @ msg                # scatter-sum via matmul, fp32 PSUM
  out = LayerNorm(agg + nf_shard)
"""

import numpy as np
import ml_dtypes

import concourse.bacc as bacc
import concourse.tile as tile
import concourse.mybir as mybir
import concourse.bass as bass

F32 = mybir.dt.float32
# float32r crashes real TRN2 (walrus codegen bug for f32r weight loads) —
# keep the transpose path in plain fp32.
F32R = mybir.dt.float32
BF16 = mybir.dt.bfloat16
I16 = mybir.dt.int16

BF = ml_dtypes.bfloat16


# ----------------------------------------------------------------------------
# Host-side preparation (indexing / layout only — no model FLOPs)
# ----------------------------------------------------------------------------

def wrap_idx(arr):
    """int16 stream -> [128, L/16] wrapped layout (replicated per 16 rows)."""
    L = arr.shape[0]
    assert L % 16 == 0
    w16 = arr.reshape(-1, 16).T.astype(np.int16)   # [16, L/16]
    return np.ascontiguousarray(np.tile(w16, (8, 1)))


def host_prep(node_features, senders, receivers, edge_features, W, b, ln_w, ln_b,
              n_cores=8):
    N, H = node_features.shape
    E = senders.shape[0]
    assert H == 128
    NPC = N // n_cores                      # nodes per core
    WPC = (NPC + 127) // 128                # windows per core
    NPC_PAD = WPC * 128

    node_features = np.asarray(node_features, np.float32)
    senders = np.asarray(senders, np.int32)
    receivers = np.asarray(receivers, np.int32)
    edge_features = np.asarray(edge_features, np.float32)

    core_of_edge = np.minimum(receivers // NPC, n_cores - 1)

    # ---- pass 1: per-core window counts ------------------------------------
    per_core = []
    cnt = np.zeros((n_cores, WPC), np.int64)
    for c in range(n_cores):
        sel = np.nonzero(core_of_edge == c)[0]
        r_loc = receivers[sel] - c * NPC
        w = r_loc >> 7
        order = np.argsort(w, kind="stable")
        sel, r_loc, w = sel[order], r_loc[order], w[order]
        cnt[c] = np.bincount(w, minlength=WPC)
        per_core.append((sel, r_loc, w))

    T_w = ((cnt.max(axis=0) + 127) // 128).astype(np.int64)  # shared all cores
    NT = int(T_w.sum())
    E_PAD = NT * 128
    tile_base = np.concatenate(([0], np.cumsum(T_w)[:-1]))

    structure = dict(N=N, H=H, E=E, NPC=NPC, WPC=WPC, NPC_PAD=NPC_PAD,
                     NT=NT, E_PAD=E_PAD, T_w=T_w, tile_base=tile_base)

    iota_row = np.broadcast_to(np.arange(128, dtype=np.float32),
                               (128, 128)).astype(BF)
    shared = {
        "W1": np.ascontiguousarray(W[0:128]).astype(BF),
        "W2": np.ascontiguousarray(W[128:256]).astype(BF),
        "W3": np.ascontiguousarray(W[256:384]).astype(BF),
        "b_col": np.asarray(b, np.float32).reshape(128, 1).copy(),
        "iota": np.ascontiguousarray(iota_row),
        "ident": np.eye(128, dtype=np.float32),
        "lnw_bc": np.broadcast_to(np.asarray(ln_w, np.float32), (128, 128)).copy(),
        "lnb_bc": np.broadcast_to(np.asarray(ln_b, np.float32), (128, 128)).copy(),
    }

    # ---- pass 2: per-core padded streams ------------------------------------
    in_maps = []
    for c in range(n_cores):
        sel, r_loc, w = per_core[c]
        Ec = sel.shape[0]
        starts = np.concatenate(([0], np.nonzero(np.diff(w))[0] + 1))
        grp_start = np.repeat(starts, np.diff(np.concatenate((starts, [Ec]))))
        j = np.arange(Ec) - grp_start
        pos = tile_base[w] * 128 + j
        rank = (r_loc & 127).astype(np.int64)

        nf_exp = np.zeros((E_PAD, 128), np.float32)
        nf_exp[pos] = node_features[senders[sel]]
        ef_pad = np.zeros((E_PAD, 128), np.float32)
        ef_pad[pos] = edge_features[sel]

        rank_arr = np.full(E_PAD, -1.0, np.float32)
        rank_arr[pos] = rank
        rankT = np.ascontiguousarray(rank_arr.reshape(NT, 128).T)   # [128, NT]

        idx_rank = np.zeros(E_PAD, np.int64)
        idx_rank[pos] = rank

        nf_shard = np.zeros((NPC_PAD, 128), np.float32)
        nf_shard[:NPC] = node_features[c * NPC:(c + 1) * NPC]
        nfT_loc = np.zeros((128, NPC_PAD), np.float32)
        nfT_loc[:, :NPC] = node_features[c * NPC:(c + 1) * NPC].T

        m = dict(shared)
        m.update({
            "nf_expT": np.ascontiguousarray(nf_exp.T).astype(BF),
            "efT": np.ascontiguousarray(ef_pad.T).astype(BF),
            "rankT": rankT,
            "idx_rank": wrap_idx(idx_rank),
            "nf_shard": nf_shard,
            "nfT_loc": np.ascontiguousarray(nfT_loc).astype(BF),
        })
        in_maps.append(m)

    return structure, in_maps


# ----------------------------------------------------------------------------
# Bass kernel builder
# ----------------------------------------------------------------------------

def _emit_ln_store(nc, wtiles, x, eps_sb, lnw_sb, lnb_sb, out_shard, w):
    """LayerNorm(x) * ln_w + ln_b -> out_shard[w*128:(w+1)*128]."""
    stats = wtiles.tile([128, 6], F32, tag="stats")
    nc.vector.bn_stats(out=stats[:], in_=x[:])
    mv = wtiles.tile([128, 2], F32, tag="mv")
    nc.vector.bn_aggr(out=mv[:], in_=stats[:])
    sd = wtiles.tile([128, 1], F32, tag="sd")
    nc.scalar.activation(
        out=sd[:], in_=mv[:, 1:2],
        func=mybir.ActivationFunctionType.Sqrt,
        bias=eps_sb[:], scale=1.0)
    rs = wtiles.tile([128, 1], F32, tag="rs")
    nc.vector.reciprocal(out=rs[:], in_=sd[:])
    xn = wtiles.tile([128, 128], F32, tag="xn")
    nc.vector.tensor_scalar(
        out=xn[:], in0=x[:], scalar1=mv[:, 0:1], scalar2=rs[:],
        op0=mybir.AluOpType.subtract, op1=mybir.AluOpType.mult)
    xw = wtiles.tile([128, 128], F32, tag="xw")
    nc.vector.tensor_mul(out=xw[:], in0=xn[:], in1=lnw_sb[:])
    ot = wtiles.tile([128, 128], F32, tag="ot")
    nc.vector.tensor_add(out=ot[:], in0=xw[:], in1=lnb_sb[:])
    nc.sync.dma_start(out=out_shard[w * 128:(w + 1) * 128, :], in_=ot[:])


def build_kernel(st, eps=1e-5, max_windows=None):
    NPC_PAD, WPC = st["NPC_PAD"], st["WPC"]
    NT, E_PAD = st["NT"], st["E_PAD"]
    T_w, tile_base = st["T_w"], st["tile_base"]
    T_MAX = int(T_w.max())
    is_eq = mybir.AluOpType.is_equal

    nc = bacc.Bacc("TRN2", target_bir_lowering=False, debug=False)

    # inputs
    nf_expT = nc.dram_tensor("nf_expT", [128, E_PAD], BF16, kind="ExternalInput")
    efT = nc.dram_tensor("efT", [128, E_PAD], BF16, kind="ExternalInput")
    rankT = nc.dram_tensor("rankT", [128, NT], F32, kind="ExternalInput")
    idx_rank = nc.dram_tensor("idx_rank", [128, E_PAD // 16], I16,
                              kind="ExternalInput")
    nfT_loc = nc.dram_tensor("nfT_loc", [128, NPC_PAD], BF16, kind="ExternalInput")
    nf_shard = nc.dram_tensor("nf_shard", [NPC_PAD, 128], F32, kind="ExternalInput")
    W1 = nc.dram_tensor("W1", [128, 128], BF16, kind="ExternalInput")
    W2 = nc.dram_tensor("W2", [128, 128], BF16, kind="ExternalInput")
    W3 = nc.dram_tensor("W3", [128, 128], BF16, kind="ExternalInput")
    b_col = nc.dram_tensor("b_col", [128, 1], F32, kind="ExternalInput")
    iota_in = nc.dram_tensor("iota", [128, 128], BF16, kind="ExternalInput")
    ident = nc.dram_tensor("ident", [128, 128], F32R, kind="ExternalInput")
    lnw_bc = nc.dram_tensor("lnw_bc", [128, 128], F32, kind="ExternalInput")
    lnb_bc = nc.dram_tensor("lnb_bc", [128, 128], F32, kind="ExternalInput")

    out_shard = nc.dram_tensor("out_shard", [NPC_PAD, 128], F32,
                               kind="ExternalOutput")

    with tile.TileContext(nc) as tc:
        with (
            tc.tile_pool(name="consts", bufs=1) as consts,
            tc.tile_pool(name="ptiles", bufs=3) as ptiles,
            tc.tile_pool(name="ppsumb", bufs=2, space="PSUM") as ppsumb,
            tc.tile_pool(name="estream", bufs=3) as estream,
            tc.tile_pool(name="gx", bufs=3) as gx,
            tc.tile_pool(name="ppsum", bufs=4, space="PSUM") as ppsum,
            tc.tile_pool(name="msgs", bufs=4) as msgs,
            tc.tile_pool(name="aggp", bufs=2, space="PSUM") as aggp,
            tc.tile_pool(name="wtiles", bufs=3) as wtiles,
        ):
            W1s = consts.tile([128, 128], BF16)
            W2s = consts.tile([128, 128], BF16)
            W3s = consts.tile([128, 128], BF16)
            iota_sb = consts.tile([128, 128], BF16)
            ident_sb = consts.tile([128, 128], F32R)
            bcol_sb = consts.tile([128, 1], F32)
            lnw_sb = consts.tile([128, 128], F32)
            lnb_sb = consts.tile([128, 128], F32)
            eps_sb = consts.tile([128, 1], F32)
            idxr_sb = consts.tile([128, E_PAD // 16], I16)
            rankT_sb = consts.tile([128, NT], F32)
            p2bT = consts.tile([128, NPC_PAD], F32R)
            for dst, src in ((W1s, W1), (W2s, W2), (W3s, W3),
                             (iota_sb, iota_in), (ident_sb, ident),
                             (bcol_sb, b_col), (lnw_sb, lnw_bc),
                             (lnb_sb, lnb_bc), (idxr_sb, idx_rank),
                             (rankT_sb, rankT)):
                nc.sync.dma_start(out=dst[:], in_=src[:])
            nc.vector.memset(eps_sb[:], eps)

            # ---- phase B: p2bT = (nf_loc @ W2 + b).T, kept in SBUF ----------
            for j0 in range(0, NPC_PAD, 512):
                k = min(512, NPC_PAD - j0)
                nfl = ptiles.tile([128, 512], BF16, tag="nfl")
                nc.sync.dma_start(out=nfl[:, :k], in_=nfT_loc[:, j0:j0 + k])
                psb = ppsumb.tile([128, 512], F32, tag="psb")
                nc.tensor.matmul(out=psb[:, :k], lhsT=W2s[:], rhs=nfl[:, :k],
                                 start=True, stop=True)
                nc.scalar.add(out=p2bT[:, j0:j0 + k], in_=psb[:, :k],
                              add=bcol_sb[:])

            # ---- edge loop --------------------------------------------------
            n_win = WPC if max_windows is None else min(max_windows, WPC)
            for w in range(n_win):
                tw = int(T_w[w])
                tb = int(tile_base[w])
                if tw == 0:
                    nf_w = wtiles.tile([128, 128], F32, tag="nfw")
                    nc.sync.dma_start(
                        out=nf_w[:], in_=nf_shard[w * 128:(w + 1) * 128, :])
                    x = wtiles.tile([128, 128], F32, tag="x")
                    nc.vector.tensor_copy(out=x[:], in_=nf_w[:])
                    _emit_ln_store(nc, wtiles, x, eps_sb, lnw_sb, lnb_sb,
                                   out_shard, w)
                    continue

                ef_sb = estream.tile([128, T_MAX * 128], BF16, tag="ef")
                nc.sync.dma_start(out=ef_sb[:, :tw * 128],
                                  in_=efT[:, tb * 128:(tb + tw) * 128])
                nfx_sb = estream.tile([128, T_MAX * 128], BF16, tag="nfx")
                nc.sync.dma_start(out=nfx_sb[:, :tw * 128],
                                  in_=nf_expT[:, tb * 128:(tb + tw) * 128])
                p2x = gx.tile([128, T_MAX * 128], F32R, tag="p2x")
                nc.gpsimd.ap_gather(
                    out_ap=p2x[:, :tw * 128].rearrange("p (n d) -> p n d", d=1),
                    in_ap=p2bT[:, w * 128:(w + 1) * 128].rearrange(
                        "p (n d) -> p n d", d=1),
                    idxs_ap=idxr_sb[:, tb * 8:(tb + tw) * 8],
                    channels=128, num_elems=128, d=1,
                    num_idxs=tw * 128)

                agg = aggp.tile([128, 128], F32, tag="agg")
                t_done = 0
                for c0 in range(0, tw, 4):
                    k = min(4, tw - c0)
                    ps = ppsum.tile([128, 4, 128], F32, tag="pp")
                    S4 = msgs.tile([128, 4, 128], BF16, tag="S")
                    for t in range(k):
                        sl = slice((c0 + t) * 128, (c0 + t + 1) * 128)
                        nc.tensor.matmul(
                            out=ps[:, t, :].bitcast(F32R),
                            lhsT=p2x[:, sl], rhs=ident_sb[:],
                            is_transpose=True,
                            start=True, stop=False, skip_group_check=True)
                        nc.tensor.matmul(
                            out=ps[:, t, :], lhsT=nfx_sb[:, sl], rhs=W1s[:],
                            start=False, stop=False, skip_group_check=True)
                        nc.tensor.matmul(
                            out=ps[:, t, :], lhsT=ef_sb[:, sl], rhs=W3s[:],
                            start=False, stop=True, skip_group_check=True)
                        nc.vector.tensor_scalar(
                            out=S4[:, t, :], in0=iota_sb[:],
                            scalar1=rankT_sb[:, tb + c0 + t:tb + c0 + t + 1],
                            scalar2=None, op0=is_eq)
                    msg = msgs.tile([128, 4, 128], BF16, tag="msg")
                    nc.scalar.activation(
                        out=msg[:, :k, :], in_=ps[:, :k, :],
                        func=mybir.ActivationFunctionType.Relu, scale=1.0)
                    for t in range(k):
                        nc.tensor.matmul(
                            out=agg[:], lhsT=S4[:, t, :], rhs=msg[:, t, :],
                            start=(t_done == 0), stop=(t_done == tw - 1),
                            skip_group_check=True)
                        t_done += 1

                nf_w = wtiles.tile([128, 128], F32, tag="nfw")
                nc.sync.dma_start(out=nf_w[:],
                                  in_=nf_shard[w * 128:(w + 1) * 128, :])
                x = wtiles.tile([128, 128], F32, tag="x")
                nc.vector.tensor_add(out=x[:], in0=agg[:], in1=nf_w[:])
                _emit_ln_store(nc, wtiles, x, eps_sb, lnw_sb, lnb_sb,
                               out_shard, w)

    nc.compile()
    return nc


# ----------------------------------------------------------------------------
# Full entry: host prep + device run + assembly
# ----------------------------------------------------------------------------

def run(node_features, senders, receivers, edge_features, W, b, ln_w, ln_b,
        n_cores=8, return_nc=False):
    from concourse.bass_utils import run_bass_kernel_spmd
    st, in_maps = host_prep(node_features, senders, receivers, edge_features,
                            W, b, ln_w, ln_b, n_cores)
    nc = build_kernel(st)
    res = run_bass_kernel_spmd(nc, in_maps, core_ids=list(range(n_cores)))
    NPC = st["NPC"]
    out = np.concatenate(
        [res.results[c]["out_shard"][:NPC] for c in range(n_cores)], axis=0)
    if return_nc:
        return out, nc, st, in_maps
    return out


# ----------------------------------------------------------------------------
# Harness entry point
# ----------------------------------------------------------------------------

def kernel(**inputs):
    """Full-input entry: shards across 8 NeuronCores internally."""
    out = run(
        node_features=np.asarray(inputs["node_features"], np.float32),
        senders=np.asarray(inputs["senders"], np.int32),
        receivers=np.asarray(inputs["receivers"], np.int32),
        edge_features=np.asarray(inputs["edge_features"], np.float32),
        W=np.asarray(inputs["W"], np.float32),
        b=np.asarray(inputs["b"], np.float32),
        ln_w=np.asarray(inputs["ln_w"], np.float32),
        ln_b=np.asarray(inputs["ln_b"], np.float32),
        n_cores=8,
    )
    return out.astype(np.float32)


# revision 8
# speedup vs baseline: 2.6372x; 1.0918x over previous
"""GNN message-passing kernel for Trainium2 (Bass/Tile), 8-core SPMD.

Sharding: edges sharded by receiver range (edge/data parallel, no collectives).
Core c owns receivers in [c*NPC, (c+1)*NPC). Host prep (pure indexing, no
FLOPs) expands nf[senders] into a per-edge stream so the device never does a
DRAM gather on the sender side.

Per core, per 128-receiver window, per 128-edge tile (all in one PSUM group):
  ps  = transpose(p2x)            # P2b[r] via window-local ap_gather (f32r)
      + nf_exp_tile.T @ W1        # bf16, host-expanded nf[senders]
      + ef_tile.T    @ W3         # bf16
  msg = relu(ps)                  # Activation engine, bf16 out
  S   = (iota == rank)            # one-hot, DVE tensor_scalar, bf16
  agg += S.T @ msg                # scatter-sum via matmul, fp32 PSUM
  out = LayerNorm(agg + nf_shard)
"""

import numpy as np
import ml_dtypes

import concourse.bacc as bacc
import concourse.tile as tile
import concourse.mybir as mybir
import concourse.bass as bass

F32 = mybir.dt.float32
# float32r crashes real TRN2 (walrus codegen bug for f32r weight loads) —
# keep the transpose path in plain fp32.
F32R = mybir.dt.float32
BF16 = mybir.dt.bfloat16
I16 = mybir.dt.int16

BF = ml_dtypes.bfloat16


# ----------------------------------------------------------------------------
# Host-side preparation (indexing / layout only — no model FLOPs)
# ----------------------------------------------------------------------------

def wrap_idx(arr):
    """int16 stream -> [128, L/16] wrapped layout (replicated per 16 rows)."""
    L = arr.shape[0]
    assert L % 16 == 0
    w16 = arr.reshape(-1, 16).T.astype(np.int16)   # [16, L/16]
    return np.ascontiguousarray(np.tile(w16, (8, 1)))


def host_prep(node_features, senders, receivers, edge_features, W, b, ln_w, ln_b,
              n_cores=8):
    N, H = node_features.shape
    E = senders.shape[0]
    assert H == 128
    NPC = N // n_cores                      # nodes per core
    WPC = (NPC + 127) // 128                # windows per core
    NPC_PAD = WPC * 128

    node_features = np.asarray(node_features, np.float32)
    senders = np.asarray(senders, np.int32)
    receivers = np.asarray(receivers, np.int32)
    edge_features = np.asarray(edge_features, np.float32)

    core_of_edge = np.minimum(receivers // NPC, n_cores - 1)

    # ---- pass 1: per-core window counts ------------------------------------
    per_core = []
    cnt = np.zeros((n_cores, WPC), np.int64)
    for c in range(n_cores):
        sel = np.nonzero(core_of_edge == c)[0]
        r_loc = receivers[sel] - c * NPC
        w = r_loc >> 7
        order = np.argsort(w, kind="stable")
        sel, r_loc, w = sel[order], r_loc[order], w[order]
        cnt[c] = np.bincount(w, minlength=WPC)
        per_core.append((sel, r_loc, w))

    T_w = ((cnt.max(axis=0) + 127) // 128).astype(np.int64)  # shared all cores
    NT = int(T_w.sum())
    E_PAD = NT * 128
    tile_base = np.concatenate(([0], np.cumsum(T_w)[:-1]))

    structure = dict(N=N, H=H, E=E, NPC=NPC, WPC=WPC, NPC_PAD=NPC_PAD,
                     NT=NT, E_PAD=E_PAD, T_w=T_w, tile_base=tile_base)

    iota_row = np.broadcast_to(np.arange(128, dtype=np.float32),
                               (128, 128)).astype(BF)
    shared = {
        "W1": np.ascontiguousarray(W[0:128]).astype(BF),
        "W2": np.ascontiguousarray(W[128:256]).astype(BF),
        "W3": np.ascontiguousarray(W[256:384]).astype(BF),
        "b_col": np.asarray(b, np.float32).reshape(128, 1).copy(),
        "iota": np.ascontiguousarray(iota_row),
        "ident": np.eye(128, dtype=np.float32).astype(BF),
        "lnw_bc": np.broadcast_to(np.asarray(ln_w, np.float32), (128, 128)).copy(),
        "lnb_bc": np.broadcast_to(np.asarray(ln_b, np.float32), (128, 128)).copy(),
    }

    # ---- pass 2: per-core padded streams ------------------------------------
    in_maps = []
    for c in range(n_cores):
        sel, r_loc, w = per_core[c]
        Ec = sel.shape[0]
        starts = np.concatenate(([0], np.nonzero(np.diff(w))[0] + 1))
        grp_start = np.repeat(starts, np.diff(np.concatenate((starts, [Ec]))))
        j = np.arange(Ec) - grp_start
        pos = tile_base[w] * 128 + j
        rank = (r_loc & 127).astype(np.int64)

        nf_exp = np.zeros((E_PAD, 128), np.float32)
        nf_exp[pos] = node_features[senders[sel]]
        ef_pad = np.zeros((E_PAD, 128), np.float32)
        ef_pad[pos] = edge_features[sel]

        rank_arr = np.full(E_PAD, -1.0, np.float32)
        rank_arr[pos] = rank
        rankT = np.ascontiguousarray(rank_arr.reshape(NT, 128).T)   # [128, NT]

        idx_rank = np.zeros(E_PAD, np.int64)
        idx_rank[pos] = rank

        nf_shard = np.zeros((NPC_PAD, 128), np.float32)
        nf_shard[:NPC] = node_features[c * NPC:(c + 1) * NPC]
        nfT_loc = np.zeros((128, NPC_PAD), np.float32)
        nfT_loc[:, :NPC] = node_features[c * NPC:(c + 1) * NPC].T

        m = dict(shared)
        m.update({
            "nf_expT": np.ascontiguousarray(nf_exp.T).astype(BF),
            "efT": np.ascontiguousarray(ef_pad.T).astype(BF),
            "rankT": rankT,
            "idx_rank": wrap_idx(idx_rank),
            "nf_shard": nf_shard,
            "nfT_loc": np.ascontiguousarray(nfT_loc).astype(BF),
        })
        in_maps.append(m)

    return structure, in_maps


# ----------------------------------------------------------------------------
# Bass kernel builder
# ----------------------------------------------------------------------------

def _emit_ln_store(nc, wtiles, x, eps_sb, lnw_sb, lnb_sb, out_shard, w):
    """LayerNorm(x) * ln_w + ln_b -> out_shard[w*128:(w+1)*128]."""
    stats = wtiles.tile([128, 6], F32, tag="stats")
    nc.vector.bn_stats(out=stats[:], in_=x[:])
    mv = wtiles.tile([128, 2], F32, tag="mv")
    nc.vector.bn_aggr(out=mv[:], in_=stats[:])
    sd = wtiles.tile([128, 1], F32, tag="sd")
    nc.scalar.activation(
        out=sd[:], in_=mv[:, 1:2],
        func=mybir.ActivationFunctionType.Sqrt,
        bias=eps_sb[:], scale=1.0)
    rs = wtiles.tile([128, 1], F32, tag="rs")
    nc.vector.reciprocal(out=rs[:], in_=sd[:])
    xn = wtiles.tile([128, 128], F32, tag="xn")
    nc.vector.tensor_scalar(
        out=xn[:], in0=x[:], scalar1=mv[:, 0:1], scalar2=rs[:],
        op0=mybir.AluOpType.subtract, op1=mybir.AluOpType.mult)
    xw = wtiles.tile([128, 128], F32, tag="xw")
    nc.vector.tensor_mul(out=xw[:], in0=xn[:], in1=lnw_sb[:])
    ot = wtiles.tile([128, 128], F32, tag="ot")
    nc.vector.tensor_add(out=ot[:], in0=xw[:], in1=lnb_sb[:])
    nc.sync.dma_start(out=out_shard[w * 128:(w + 1) * 128, :], in_=ot[:])


def build_kernel(st, eps=1e-5, max_windows=None):
    NPC_PAD, WPC = st["NPC_PAD"], st["WPC"]
    NT, E_PAD = st["NT"], st["E_PAD"]
    T_w, tile_base = st["T_w"], st["tile_base"]
    T_MAX = int(T_w.max())
    is_eq = mybir.AluOpType.is_equal

    nc = bacc.Bacc("TRN2", target_bir_lowering=False, debug=False)

    # inputs
    nf_expT = nc.dram_tensor("nf_expT", [128, E_PAD], BF16, kind="ExternalInput")
    efT = nc.dram_tensor("efT", [128, E_PAD], BF16, kind="ExternalInput")
    rankT = nc.dram_tensor("rankT", [128, NT], F32, kind="ExternalInput")
    idx_rank = nc.dram_tensor("idx_rank", [128, E_PAD // 16], I16,
                              kind="ExternalInput")
    nfT_loc = nc.dram_tensor("nfT_loc", [128, NPC_PAD], BF16, kind="ExternalInput")
    nf_shard = nc.dram_tensor("nf_shard", [NPC_PAD, 128], F32, kind="ExternalInput")
    W1 = nc.dram_tensor("W1", [128, 128], BF16, kind="ExternalInput")
    W2 = nc.dram_tensor("W2", [128, 128], BF16, kind="ExternalInput")
    W3 = nc.dram_tensor("W3", [128, 128], BF16, kind="ExternalInput")
    b_col = nc.dram_tensor("b_col", [128, 1], F32, kind="ExternalInput")
    iota_in = nc.dram_tensor("iota", [128, 128], BF16, kind="ExternalInput")
    ident = nc.dram_tensor("ident", [128, 128], BF16, kind="ExternalInput")
    lnw_bc = nc.dram_tensor("lnw_bc", [128, 128], F32, kind="ExternalInput")
    lnb_bc = nc.dram_tensor("lnb_bc", [128, 128], F32, kind="ExternalInput")

    out_shard = nc.dram_tensor("out_shard", [NPC_PAD, 128], F32,
                               kind="ExternalOutput")

    with tile.TileContext(nc) as tc:
        with (
            tc.tile_pool(name="consts", bufs=1) as consts,
            tc.tile_pool(name="ptiles", bufs=3) as ptiles,
            tc.tile_pool(name="ppsumb", bufs=2, space="PSUM") as ppsumb,
            tc.tile_pool(name="estream", bufs=3) as estream,
            tc.tile_pool(name="gx", bufs=3) as gx,
            tc.tile_pool(name="ppsum", bufs=4, space="PSUM") as ppsum,
            tc.tile_pool(name="msgs", bufs=4) as msgs,
            tc.tile_pool(name="aggp", bufs=2, space="PSUM") as aggp,
            tc.tile_pool(name="wtiles", bufs=3) as wtiles,
        ):
            W1s = consts.tile([128, 128], BF16)
            W2s = consts.tile([128, 128], BF16)
            W3s = consts.tile([128, 128], BF16)
            iota_sb = consts.tile([128, 128], BF16)
            ident_sb = consts.tile([128, 128], BF16)
            bcol_sb = consts.tile([128, 1], F32)
            lnw_sb = consts.tile([128, 128], F32)
            lnb_sb = consts.tile([128, 128], F32)
            eps_sb = consts.tile([128, 1], F32)
            idxr_sb = consts.tile([128, E_PAD // 16], I16)
            rankT_sb = consts.tile([128, NT], F32)
            p2bT = consts.tile([128, NPC_PAD], F32R)
            for dst, src in ((W1s, W1), (W2s, W2), (W3s, W3),
                             (iota_sb, iota_in), (ident_sb, ident),
                             (bcol_sb, b_col), (lnw_sb, lnw_bc),
                             (lnb_sb, lnb_bc), (idxr_sb, idx_rank),
                             (rankT_sb, rankT)):
                nc.sync.dma_start(out=dst[:], in_=src[:])
            nc.vector.memset(eps_sb[:], eps)

            # ---- phase B: p2bT = (nf_loc @ W2 + b).T, kept in SBUF ----------
            for j0 in range(0, NPC_PAD, 512):
                k = min(512, NPC_PAD - j0)
                nfl = ptiles.tile([128, 512], BF16, tag="nfl")
                nc.sync.dma_start(out=nfl[:, :k], in_=nfT_loc[:, j0:j0 + k])
                psb = ppsumb.tile([128, 512], F32, tag="psb")
                nc.tensor.matmul(out=psb[:, :k], lhsT=W2s[:], rhs=nfl[:, :k],
                                 start=True, stop=True)
                nc.scalar.add(out=p2bT[:, j0:j0 + k], in_=psb[:, :k],
                              add=bcol_sb[:])

            # ---- edge loop --------------------------------------------------
            n_win = WPC if max_windows is None else min(max_windows, WPC)
            for w in range(n_win):
                tw = int(T_w[w])
                tb = int(tile_base[w])
                if tw == 0:
                    nf_w = wtiles.tile([128, 128], F32, tag="nfw")
                    nc.sync.dma_start(
                        out=nf_w[:], in_=nf_shard[w * 128:(w + 1) * 128, :])
                    x = wtiles.tile([128, 128], F32, tag="x")
                    nc.vector.tensor_copy(out=x[:], in_=nf_w[:])
                    _emit_ln_store(nc, wtiles, x, eps_sb, lnw_sb, lnb_sb,
                                   out_shard, w)
                    continue

                ef_sb = estream.tile([128, T_MAX * 128], BF16, tag="ef")
                nc.sync.dma_start(out=ef_sb[:, :tw * 128],
                                  in_=efT[:, tb * 128:(tb + tw) * 128])
                nfx_sb = estream.tile([128, T_MAX * 128], BF16, tag="nfx")
                nc.sync.dma_start(out=nfx_sb[:, :tw * 128],
                                  in_=nf_expT[:, tb * 128:(tb + tw) * 128])
                p2x = gx.tile([128, T_MAX * 128], F32R, tag="p2x")
                nc.gpsimd.ap_gather(
                    out_ap=p2x[:, :tw * 128].rearrange("p (n d) -> p n d", d=1),
                    in_ap=p2bT[:, w * 128:(w + 1) * 128].rearrange(
                        "p (n d) -> p n d", d=1),
                    idxs_ap=idxr_sb[:, tb * 8:(tb + tw) * 8],
                    channels=128, num_elems=128, d=1,
                    num_idxs=tw * 128)
                # bf16 view of p2x: odd lanes of the little-endian fp32 words
                # are the truncated-bf16 values -> 1 cyc/row PE transpose via
                # a normal matmul against the bf16 identity.
                p2x_bf = p2x[:].bitcast(BF16).rearrange(
                    "p (n two) -> p n two", two=2)

                agg = aggp.tile([128, 128], F32, tag="agg")
                t_done = 0
                for c0 in range(0, tw, 4):
                    k = min(4, tw - c0)
                    ps = ppsum.tile([128, 4, 128], F32, tag="pp")
                    S4 = msgs.tile([128, 4, 128], BF16, tag="S")
                    for t in range(k):
                        sl = slice((c0 + t) * 128, (c0 + t + 1) * 128)
                        nc.tensor.matmul(
                            out=ps[:, t, :],
                            lhsT=p2x_bf[:, sl, 1],
                            rhs=ident_sb[:],
                            start=True, stop=False, skip_group_check=True)
                        nc.tensor.matmul(
                            out=ps[:, t, :], lhsT=nfx_sb[:, sl], rhs=W1s[:],
                            start=False, stop=False, skip_group_check=True)
                        nc.tensor.matmul(
                            out=ps[:, t, :], lhsT=ef_sb[:, sl], rhs=W3s[:],
                            start=False, stop=True, skip_group_check=True)
                        nc.vector.tensor_scalar(
                            out=S4[:, t, :], in0=iota_sb[:],
                            scalar1=rankT_sb[:, tb + c0 + t:tb + c0 + t + 1],
                            scalar2=None, op0=is_eq)
                    msg = msgs.tile([128, 4, 128], BF16, tag="msg")
                    nc.scalar.activation(
                        out=msg[:, :k, :], in_=ps[:, :k, :],
                        func=mybir.ActivationFunctionType.Relu, scale=1.0)
                    for t in range(k):
                        nc.tensor.matmul(
                            out=agg[:], lhsT=S4[:, t, :], rhs=msg[:, t, :],
                            start=(t_done == 0), stop=(t_done == tw - 1),
                            skip_group_check=True)
                        t_done += 1

                nf_w = wtiles.tile([128, 128], F32, tag="nfw")
                nc.sync.dma_start(out=nf_w[:],
                                  in_=nf_shard[w * 128:(w + 1) * 128, :])
                x = wtiles.tile([128, 128], F32, tag="x")
                nc.vector.tensor_add(out=x[:], in0=agg[:], in1=nf_w[:])
                _emit_ln_store(nc, wtiles, x, eps_sb, lnw_sb, lnb_sb,
                               out_shard, w)

    nc.compile()
    return nc


# ----------------------------------------------------------------------------
# Full entry: host prep + device run + assembly
# ----------------------------------------------------------------------------

def run(node_features, senders, receivers, edge_features, W, b, ln_w, ln_b,
        n_cores=8, return_nc=False):
    from concourse.bass_utils import run_bass_kernel_spmd
    st, in_maps = host_prep(node_features, senders, receivers, edge_features,
                            W, b, ln_w, ln_b, n_cores)
    nc = build_kernel(st)
    res = run_bass_kernel_spmd(nc, in_maps, core_ids=list(range(n_cores)))
    NPC = st["NPC"]
    out = np.concatenate(
        [res.results[c]["out_shard"][:NPC] for c in range(n_cores)], axis=0)
    if return_nc:
        return out, nc, st, in_maps
    return out


# ----------------------------------------------------------------------------
# Harness entry point
# ----------------------------------------------------------------------------

def kernel(**inputs):
    """Full-input entry: shards across 8 NeuronCores internally."""
    out = run(
        node_features=np.asarray(inputs["node_features"], np.float32),
        senders=np.asarray(inputs["senders"], np.int32),
        receivers=np.asarray(inputs["receivers"], np.int32),
        edge_features=np.asarray(inputs["edge_features"], np.float32),
        W=np.asarray(inputs["W"], np.float32),
        b=np.asarray(inputs["b"], np.float32),
        ln_w=np.asarray(inputs["ln_w"], np.float32),
        ln_b=np.asarray(inputs["ln_b"], np.float32),
        n_cores=8,
    )
    return out.astype(np.float32)


# revision 16
# speedup vs baseline: 2.8778x; 1.0912x over previous
"""GNN message-passing kernel for Trainium2 (Bass/Tile), 8-core SPMD.

Sharding: edges sharded by receiver range (edge/data parallel, no collectives).
Core c owns receivers in [c*NPC, (c+1)*NPC). Host prep (pure indexing, no
FLOPs) expands nf[senders] into a per-edge stream so the device never does a
DRAM gather on the sender side.

Per core, per 128-receiver window, per 128-edge tile (all in one PSUM group):
  ps  = transpose(p2x)            # P2b[r] via window-local ap_gather (f32r)
      + nf_exp_tile.T @ W1        # bf16, host-expanded nf[senders]
      + ef_tile.T    @ W3         # bf16
  msg = relu(ps)                  # Activation engine, bf16 out
  S   = (iota == rank)            # one-hot, DVE tensor_scalar, bf16
  agg += S.T @ msg                # scatter-sum via matmul, fp32 PSUM
  out = LayerNorm(agg + nf_shard)
"""

import numpy as np
import ml_dtypes

import concourse.bacc as bacc
import concourse.tile as tile
import concourse.mybir as mybir
import concourse.bass as bass

F32 = mybir.dt.float32
# float32r crashes real TRN2 (walrus codegen bug for f32r weight loads) —
# keep the transpose path in plain fp32.
F32R = mybir.dt.float32
BF16 = mybir.dt.bfloat16
I16 = mybir.dt.int16

BF = ml_dtypes.bfloat16


# ----------------------------------------------------------------------------
# Host-side preparation (indexing / layout only — no model FLOPs)
# ----------------------------------------------------------------------------

def wrap_idx(arr):
    """int16 stream -> [128, L/16] wrapped layout (replicated per 16 rows)."""
    L = arr.shape[0]
    assert L % 16 == 0
    w16 = arr.reshape(-1, 16).T.astype(np.int16)   # [16, L/16]
    return np.ascontiguousarray(np.tile(w16, (8, 1)))


def host_prep(node_features, senders, receivers, edge_features, W, b, ln_w, ln_b,
              n_cores=8):
    N, H = node_features.shape
    E = senders.shape[0]
    assert H == 128
    NPC = N // n_cores                      # nodes per core
    WPC = (NPC + 127) // 128                # windows per core
    NPC_PAD = WPC * 128

    node_features = np.asarray(node_features, np.float32)
    senders = np.asarray(senders, np.int32)
    receivers = np.asarray(receivers, np.int32)
    edge_features = np.asarray(edge_features, np.float32)

    core_of_edge = np.minimum(receivers // NPC, n_cores - 1)

    # ---- pass 1: per-core window counts ------------------------------------
    per_core = []
    cnt = np.zeros((n_cores, WPC), np.int64)
    for c in range(n_cores):
        sel = np.nonzero(core_of_edge == c)[0]
        r_loc = receivers[sel] - c * NPC
        w = r_loc >> 7
        order = np.argsort(w, kind="stable")
        sel, r_loc, w = sel[order], r_loc[order], w[order]
        cnt[c] = np.bincount(w, minlength=WPC)
        per_core.append((sel, r_loc, w))

    T_w = ((cnt.max(axis=0) + 127) // 128).astype(np.int64)  # shared all cores
    NT = int(T_w.sum())
    E_PAD = NT * 128
    tile_base = np.concatenate(([0], np.cumsum(T_w)[:-1]))

    structure = dict(N=N, H=H, E=E, NPC=NPC, WPC=WPC, NPC_PAD=NPC_PAD,
                     NT=NT, E_PAD=E_PAD, T_w=T_w, tile_base=tile_base)

    iota_row = np.broadcast_to(np.arange(128, dtype=np.float32),
                               (128, 128)).astype(BF)
    shared = {
        "W1": np.ascontiguousarray(W[0:128]).astype(BF),
        "W2": np.ascontiguousarray(W[128:256]).astype(BF),
        "W3": np.ascontiguousarray(W[256:384]).astype(BF),
        "b_col": np.asarray(b, np.float32).reshape(128, 1).copy(),
        "iota": np.ascontiguousarray(iota_row),
        "ident": np.eye(128, dtype=np.float32).astype(BF),
        "lnw_bc": np.broadcast_to(np.asarray(ln_w, np.float32), (128, 128)).copy(),
        "lnb_bc": np.broadcast_to(np.asarray(ln_b, np.float32), (128, 128)).copy(),
    }

    # ---- pass 2: per-core padded streams ------------------------------------
    in_maps = []
    for c in range(n_cores):
        sel, r_loc, w = per_core[c]
        Ec = sel.shape[0]
        starts = np.concatenate(([0], np.nonzero(np.diff(w))[0] + 1))
        grp_start = np.repeat(starts, np.diff(np.concatenate((starts, [Ec]))))
        j = np.arange(Ec) - grp_start
        pos = tile_base[w] * 128 + j
        rank = (r_loc & 127).astype(np.int64)

        nf_exp = np.zeros((E_PAD, 128), np.float32)
        nf_exp[pos] = node_features[senders[sel]]
        ef_pad = np.zeros((E_PAD, 128), np.float32)
        ef_pad[pos] = edge_features[sel]

        rank_arr = np.full(E_PAD, -1.0, np.float32)
        rank_arr[pos] = rank
        rankT = np.ascontiguousarray(rank_arr.reshape(NT, 128).T)   # [128, NT]

        idx_rank = np.zeros(E_PAD, np.int64)
        idx_rank[pos] = rank

        nf_shard = np.zeros((NPC_PAD, 128), np.float32)
        nf_shard[:NPC] = node_features[c * NPC:(c + 1) * NPC]
        nfT_loc = np.zeros((128, NPC_PAD), np.float32)
        nfT_loc[:, :NPC] = node_features[c * NPC:(c + 1) * NPC].T

        m = dict(shared)
        m.update({
            "nf_expT": np.ascontiguousarray(nf_exp.T).astype(BF),
            "efT": np.ascontiguousarray(ef_pad.T).astype(BF),
            "rankT": rankT,
            "idx_rank": wrap_idx(idx_rank),
            "nf_shard": nf_shard,
            "nfT_loc": np.ascontiguousarray(nfT_loc).astype(BF),
        })
        in_maps.append(m)

    return structure, in_maps


# ----------------------------------------------------------------------------
# Bass kernel builder
# ----------------------------------------------------------------------------

def _emit_ln_store(nc, wtiles, x, eps_sb, lnw_sb, lnb_sb, out_shard, w):
    """LayerNorm(x) * ln_w + ln_b -> out_shard[w*128:(w+1)*128].

    The store goes out on the Activation DGE queue so it cannot head-of-line
    block the SP queue that prefetches the next windows' edge streams.
    """
    stats = wtiles.tile([128, 6], F32, tag="stats")
    nc.vector.bn_stats(out=stats[:], in_=x[:])
    mv = wtiles.tile([128, 2], F32, tag="mv")
    nc.vector.bn_aggr(out=mv[:], in_=stats[:])
    sd = wtiles.tile([128, 1], F32, tag="sd")
    nc.scalar.activation(
        out=sd[:], in_=mv[:, 1:2],
        func=mybir.ActivationFunctionType.Sqrt,
        bias=eps_sb[:], scale=1.0)
    rs = wtiles.tile([128, 1], F32, tag="rs")
    nc.vector.reciprocal(out=rs[:], in_=sd[:])
    xn = wtiles.tile([128, 128], F32, tag="xn")
    nc.vector.tensor_scalar(
        out=xn[:], in0=x[:], scalar1=mv[:, 0:1], scalar2=rs[:],
        op0=mybir.AluOpType.subtract, op1=mybir.AluOpType.mult)
    xw = wtiles.tile([128, 128], F32, tag="xw")
    nc.vector.tensor_mul(out=xw[:], in0=xn[:], in1=lnw_sb[:])
    ot = wtiles.tile([128, 128], F32, tag="ot")
    nc.vector.tensor_add(out=ot[:], in0=xw[:], in1=lnb_sb[:])
    nc.scalar.dma_start(out=out_shard[w * 128:(w + 1) * 128, :], in_=ot[:])


def build_kernel(st, eps=1e-5, max_windows=None):
    NPC_PAD, WPC = st["NPC_PAD"], st["WPC"]
    NT, E_PAD = st["NT"], st["E_PAD"]
    T_w, tile_base = st["T_w"], st["tile_base"]
    T_MAX = int(T_w.max())
    is_eq = mybir.AluOpType.is_equal

    nc = bacc.Bacc("TRN2", target_bir_lowering=False, debug=False)

    # inputs
    nf_expT = nc.dram_tensor("nf_expT", [128, E_PAD], BF16, kind="ExternalInput")
    efT = nc.dram_tensor("efT", [128, E_PAD], BF16, kind="ExternalInput")
    rankT = nc.dram_tensor("rankT", [128, NT], F32, kind="ExternalInput")
    idx_rank = nc.dram_tensor("idx_rank", [128, E_PAD // 16], I16,
                              kind="ExternalInput")
    nfT_loc = nc.dram_tensor("nfT_loc", [128, NPC_PAD], BF16, kind="ExternalInput")
    nf_shard = nc.dram_tensor("nf_shard", [NPC_PAD, 128], F32, kind="ExternalInput")
    W1 = nc.dram_tensor("W1", [128, 128], BF16, kind="ExternalInput")
    W2 = nc.dram_tensor("W2", [128, 128], BF16, kind="ExternalInput")
    W3 = nc.dram_tensor("W3", [128, 128], BF16, kind="ExternalInput")
    b_col = nc.dram_tensor("b_col", [128, 1], F32, kind="ExternalInput")
    iota_in = nc.dram_tensor("iota", [128, 128], BF16, kind="ExternalInput")
    ident = nc.dram_tensor("ident", [128, 128], BF16, kind="ExternalInput")
    lnw_bc = nc.dram_tensor("lnw_bc", [128, 128], F32, kind="ExternalInput")
    lnb_bc = nc.dram_tensor("lnb_bc", [128, 128], F32, kind="ExternalInput")

    out_shard = nc.dram_tensor("out_shard", [NPC_PAD, 128], F32,
                               kind="ExternalOutput")

    with tile.TileContext(nc) as tc:
        with (
            tc.tile_pool(name="consts", bufs=1) as consts,
            tc.tile_pool(name="ptiles", bufs=3) as ptiles,
            tc.tile_pool(name="estream", bufs=4) as estream,
            tc.tile_pool(name="gx", bufs=4) as gx,
            tc.tile_pool(name="ppsum", bufs=6, space="PSUM") as ppsum,
            tc.tile_pool(name="msgs", bufs=4) as msgs,
            tc.tile_pool(name="aggp", bufs=2, space="PSUM") as aggp,
            tc.tile_pool(name="wtiles", bufs=3) as wtiles,
        ):
            W1s = consts.tile([128, 128], BF16)
            W2s = consts.tile([128, 128], BF16)
            W3s = consts.tile([128, 128], BF16)
            iota_sb = consts.tile([128, 128], BF16)
            ident_sb = consts.tile([128, 128], BF16)
            bcol_sb = consts.tile([128, 1], F32)
            lnw_sb = consts.tile([128, 128], F32)
            lnb_sb = consts.tile([128, 128], F32)
            eps_sb = consts.tile([128, 1], F32)
            idxr_sb = consts.tile([128, E_PAD // 16], I16)
            rankT_sb = consts.tile([128, NT], F32)
            p2bT = consts.tile([128, NPC_PAD], F32R)
            for dst, src in ((W1s, W1), (W2s, W2), (W3s, W3),
                             (iota_sb, iota_in), (ident_sb, ident),
                             (bcol_sb, b_col), (lnw_sb, lnw_bc),
                             (lnb_sb, lnb_bc), (idxr_sb, idx_rank),
                             (rankT_sb, rankT)):
                nc.sync.dma_start(out=dst[:], in_=src[:])
            nc.vector.memset(eps_sb[:], eps)

            # ---- phase B: p2bT = (nf_loc @ W2 + b).T, kept in SBUF ----------
            for j0 in range(0, NPC_PAD, 512):
                k = min(512, NPC_PAD - j0)
                nfl = ptiles.tile([128, 512], BF16, tag="nfl")
                nc.sync.dma_start(out=nfl[:, :k], in_=nfT_loc[:, j0:j0 + k])
                psb = ppsum.tile([128, 4, 128], F32, tag="pp")
                psb_flat = psb[:].rearrange("p a b -> p (a b)")
                nc.tensor.matmul(out=psb_flat[:, :k], lhsT=W2s[:],
                                 rhs=nfl[:, :k], start=True, stop=True)
                nc.scalar.add(out=p2bT[:, j0:j0 + k], in_=psb_flat[:, :k],
                              add=bcol_sb[:])

            # ---- edge loop --------------------------------------------------
            n_win = WPC if max_windows is None else min(max_windows, WPC)
            for w in range(n_win):
                tw = int(T_w[w])
                tb = int(tile_base[w])
                if tw == 0:
                    nf_w = wtiles.tile([128, 128], F32, tag="nfw")
                    nc.sync.dma_start(
                        out=nf_w[:], in_=nf_shard[w * 128:(w + 1) * 128, :])
                    x = wtiles.tile([128, 128], F32, tag="x")
                    nc.vector.tensor_copy(out=x[:], in_=nf_w[:])
                    _emit_ln_store(nc, wtiles, x, eps_sb, lnw_sb, lnb_sb,
                                   out_shard, w)
                    continue

                ef_sb = estream.tile([128, T_MAX * 128], BF16, tag="ef")
                nc.sync.dma_start(out=ef_sb[:, :tw * 128],
                                  in_=efT[:, tb * 128:(tb + tw) * 128])
                nfx_sb = estream.tile([128, T_MAX * 128], BF16, tag="nfx")
                nc.sync.dma_start(out=nfx_sb[:, :tw * 128],
                                  in_=nf_expT[:, tb * 128:(tb + tw) * 128])
                nf_w = wtiles.tile([128, 128], F32, tag="nfw")
                nc.scalar.dma_start(out=nf_w[:],
                                    in_=nf_shard[w * 128:(w + 1) * 128, :])
                p2x = gx.tile([128, T_MAX * 128], F32R, tag="p2x")
                nc.gpsimd.ap_gather(
                    out_ap=p2x[:, :tw * 128].rearrange("p (n d) -> p n d", d=1),
                    in_ap=p2bT[:, w * 128:(w + 1) * 128].rearrange(
                        "p (n d) -> p n d", d=1),
                    idxs_ap=idxr_sb[:, tb * 8:(tb + tw) * 8],
                    channels=128, num_elems=128, d=1,
                    num_idxs=tw * 128)
                # bf16 view of p2x: odd lanes of the little-endian fp32 words
                # are the truncated-bf16 values -> 1 cyc/row PE transpose via
                # a normal matmul against the bf16 identity.
                p2x_bf = p2x[:].bitcast(BF16).rearrange(
                    "p (n two) -> p n two", two=2)

                agg = aggp.tile([128, 128], F32, tag="agg")
                t_done = 0
                for c0 in range(0, tw, 4):
                    k = min(4, tw - c0)
                    ps = ppsum.tile([128, 4, 128], F32, tag="pp")
                    S4 = msgs.tile([128, 4, 128], BF16, tag="S")
                    for t in range(k):
                        sl = slice((c0 + t) * 128, (c0 + t + 1) * 128)
                        nc.tensor.matmul(
                            out=ps[:, t, :],
                            lhsT=p2x_bf[:, sl, 1],
                            rhs=ident_sb[:],
                            start=True, stop=False, skip_group_check=True)
                        nc.tensor.matmul(
                            out=ps[:, t, :], lhsT=nfx_sb[:, sl], rhs=W1s[:],
                            start=False, stop=False, skip_group_check=True)
                        nc.tensor.matmul(
                            out=ps[:, t, :], lhsT=ef_sb[:, sl], rhs=W3s[:],
                            start=False, stop=True, skip_group_check=True)
                        nc.vector.tensor_scalar(
                            out=S4[:, t, :], in0=iota_sb[:],
                            scalar1=rankT_sb[:, tb + c0 + t:tb + c0 + t + 1],
                            scalar2=None, op0=is_eq)
                    msg = msgs.tile([128, 4, 128], BF16, tag="msg")
                    nc.scalar.activation(
                        out=msg[:, :k, :], in_=ps[:, :k, :],
                        func=mybir.ActivationFunctionType.Relu, scale=1.0)
                    for t in range(k):
                        nc.tensor.matmul(
                            out=agg[:], lhsT=S4[:, t, :], rhs=msg[:, t, :],
                            start=(t_done == 0), stop=(t_done == tw - 1),
                            skip_group_check=True)
                        t_done += 1

                x = wtiles.tile([128, 128], F32, tag="x")
                nc.vector.tensor_add(out=x[:], in0=agg[:], in1=nf_w[:])
                _emit_ln_store(nc, wtiles, x, eps_sb, lnw_sb, lnb_sb,
                               out_shard, w)

    nc.compile()
    return nc


# ----------------------------------------------------------------------------
# Full entry: host prep + device run + assembly
# ----------------------------------------------------------------------------

def run(node_features, senders, receivers, edge_features, W, b, ln_w, ln_b,
        n_cores=8, return_nc=False):
    from concourse.bass_utils import run_bass_kernel_spmd
    st, in_maps = host_prep(node_features, senders, receivers, edge_features,
                            W, b, ln_w, ln_b, n_cores)
    nc = build_kernel(st)
    res = run_bass_kernel_spmd(nc, in_maps, core_ids=list(range(n_cores)))
    NPC = st["NPC"]
    out = np.concatenate(
        [res.results[c]["out_shard"][:NPC] for c in range(n_cores)], axis=0)
    if return_nc:
        return out, nc, st, in_maps
    return out


# ----------------------------------------------------------------------------
# Harness entry point
# ----------------------------------------------------------------------------

def kernel(**inputs):
    """Full-input entry: shards across 8 NeuronCores internally."""
    out = run(
        node_features=np.asarray(inputs["node_features"], np.float32),
        senders=np.asarray(inputs["senders"], np.int32),
        receivers=np.asarray(inputs["receivers"], np.int32),
        edge_features=np.asarray(inputs["edge_features"], np.float32),
        W=np.asarray(inputs["W"], np.float32),
        b=np.asarray(inputs["b"], np.float32),
        ln_w=np.asarray(inputs["ln_w"], np.float32),
        ln_b=np.asarray(inputs["ln_b"], np.float32),
        n_cores=8,
    )
    return out.astype(np.float32)
